# revision 9
# baseline (speedup 1.0000x reference)
"""Multi-head attention Bass/Tile kernel for Trainium2, 8-core SPMD.

Problem: B=8, S=1024, D=1024, H=16 (head dim 64) attention that returns
both the attention output [B,S,D] and the softmax probabilities
[B,H,S,S].

Sharding: data-parallel over batch -- core b computes batch element b.

Per-core design (one batch element):
  - The host passes hs[b].T (``hsT`` [D,S]) so the Q/K projections can
    produce QT/KT in transposed [d, s] layout directly (the PE contracts
    over the partition dim of both operands, so hs always appears
    transposed; transposing on the host is free).
  - scores are computed transposed: scoresT[sk, sq] = K Q^T via
    lhsT=KT_h, rhs=QT_h (contraction over the 64-dim head axis).
  - exp on the scalar engine with the 1/sqrt(64) scale folded in.
  - V is kept in natural [sk, dv] layout with a ones column appended per
    head, so each context matmul (lhsT=V_h|1, rhs=expT_h) also produces
    the softmax denominators in psum row 64.
  - probs output is written to DRAM as [H, sk, sq] (contiguous stores);
    the host transposes to [H, sq, sk] while gathering (f32 DMA
    transpose does not exist on TRN2, and recomputing scores in the
    other orientation would double the scalar-engine exp work).
  - normalized ctx^T is staged through a DRAM scratch and re-read
    pair-packed so the out-projection runs K=128 matmuls and produces
    the attention output in natural [s, d] layout directly.
  - All matmuls run as float32r (full PE rate at free-dim >= 256).
    Walrus requires fp32r matmul operands to be *produced* rounded, so
    every matmul-feeding tile is dtype float32r (DMA loads bitcast the
    f32 DRAM side; ACT/DVE producers write f32r directly).
"""

import os

os.environ.setdefault("MYCRO_LOCAL_CACHE", "1")

from contextlib import ExitStack

import numpy as np

import concourse.bass as bass  # noqa: F401  (bass must import before tile)
import concourse.mybir as mybir
import concourse.tile as tile
from concourse import bacc
from concourse.bass_utils import run_bass_kernel_spmd

F32 = mybir.dt.float32
F32R = mybir.dt.float32r
AF = mybir.ActivationFunctionType

B, S, D, H, HD = 8, 1024, 1024, 16, 64
P = 128
NCH = D // P  # 8 chunks of 128 along d or s
HF = 512  # fp32 psum bank free-dim limit
NHALF = S // HF  # 2
NCORES = 8


def _emit(tc, io):
    nc = tc.nc
    hsT, wq, bq, wk, bk, wv, bv, wo, bo = (
        io["hsT"], io["Wq"], io["bq"], io["Wk"], io["bk"],
        io["Wv"], io["bv"], io["Wo"], io["bo"],
    )
    out, probsT = io["out"], io["probsT"]
    R = F32R

    with ExitStack() as top:
        const = top.enter_context(tc.tile_pool(name="const", bufs=1))
        ones_f = const.tile([P, P], F32, tag="ones_f", name="ones_f")
        nc.vector.memset(ones_f[:, :], 1.0)
        # fp32r view of ones for rank-1 bias / broadcast matmuls
        # (memset cannot write f32r directly; a DVE copy can)
        ones_r = const.tile([P, P], R, tag="ones_r", name="ones_r")
        nc.vector.tensor_copy(ones_r[:, :], ones_f[:, :])
        bq_sb = const.tile([P, NCH], F32, tag="bq_sb", name="bq_sb")
        nc.sync.dma_start(bq_sb[:, :], bq.rearrange("(c p) -> p c", p=P))
        bk_sb = const.tile([P, NCH], F32, tag="bk_sb", name="bk_sb")
        nc.sync.dma_start(bk_sb[:, :], bk.rearrange("(c p) -> p c", p=P))
        bv_row = const.tile([1, D], R, tag="bv_row", name="bv_row")
        nc.sync.dma_start(bv_row[:, :], bv.rearrange("(o d) -> o d", o=1).bitcast(R))
        bo_row = const.tile([1, D], R, tag="bo_row", name="bo_row")
        nc.sync.dma_start(bo_row[:, :], bo.rearrange("(o d) -> o d", o=1).bitcast(R))

        # PSUM pools all at top level: 2 + 4 + 1 + 1 = 8 banks.
        psA = top.enter_context(tc.tile_pool(name="psA", bufs=2, space="PSUM"))
        ps_sc = top.enter_context(tc.tile_pool(name="ps_sc", bufs=2, space="PSUM"))
        ps_ctx = top.enter_context(tc.tile_pool(name="ps_ctx", bufs=1, space="PSUM"))
        ps_bc = top.enter_context(tc.tile_pool(name="ps_bc", bufs=1, space="PSUM"))

        # Normalized ctx^T staged through DRAM ([dc, s]); re-read in the
        # out-projection as pair-packed [128, S] chunks (keeps SBUF small
        # and the out-proj matmuls at K=128).
        ctx_dram = nc.dram_tensor("ctx_scratch", [D, S], R, kind="Internal").ap()

        with ExitStack() as mid:
            qkvp = mid.enter_context(tc.tile_pool(name="qkvp", bufs=1))
            vt = [qkvp.tile([P, H, HD + 1], R, tag=f"v{i}", name=f"v{i}")
                  for i in range(NCH)]
            for i in range(NCH):
                # ones column per head (f32r via DVE copy from f32 ones)
                nc.vector.tensor_copy(
                    vt[i][:, :, HD : HD + 1],
                    ones_f[:, 0:H].rearrange("p (h o) -> p h o", o=1),
                )
            qktp = mid.enter_context(tc.tile_pool(name="qktp", bufs=1))
            qt = [None] * NCH
            kt = [None] * NCH

            with ExitStack() as ld:
                ldp = ld.enter_context(tc.tile_pool(name="ldp", bufs=1))
                hst = []
                for c in range(NCH):
                    t = ldp.tile([P, S], R, tag=f"hst{c}", bufs=1, name=f"hst{c}")
                    nc.sync.dma_start(t[:, :], hsT[c * P : (c + 1) * P, :].bitcast(R))
                    hst.append(t)

                # --- V projection: V[sk, dv] = hs @ Wv + bv ---
                wv_r = wv.rearrange("(c p) d -> p c d", p=P).bitcast(R)
                for n in range(NHALF):
                    wvt = ldp.tile([P, NCH, HF], R, tag="wv", bufs=1, name="wvt")
                    nc.sync.dma_start(wvt[:, :, :], wv_r[:, :, n * HF : (n + 1) * HF])
                    for i in range(NCH):
                        ps = psA.tile([P, HF], F32, tag="ps", name="ps_v")
                        for c in range(NCH):
                            nc.tensor.matmul(
                                ps[:, :],
                                hst[c][:, i * P : (i + 1) * P],
                                wvt[:, c, :],
                                start=(c == 0),
                                stop=False,
                            )
                        # bias as a rank-1 (ones ⊗ bv) accumulation
                        nc.tensor.matmul(
                            ps[:, :],
                            ones_r[0:1, :],
                            bv_row[0:1, n * HF : (n + 1) * HF],
                            start=False,
                            stop=True,
                        )
                        # evict on ScalarE (keeps VectorE free for the
                        # attention-phase normalize work)
                        nc.scalar.copy(
                            vt[i][:, n * 8 : (n + 1) * 8, 0:HD],
                            ps.rearrange("p (h e) -> p h e", e=HD),
                        )

                # --- Q/K projections into transposed [do, s] layout ---
                wq_r = wq.rearrange("(c p) (j q) -> p c j q", p=P, q=P).bitcast(R)
                wk_r = wk.rearrange("(c p) (j q) -> p c j q", p=P, q=P).bitcast(R)
                for j in range(NCH):
                    for (w_r, b_sb, dst, tag) in (
                        (wq_r, bq_sb, qt, "qtile"),
                        (wk_r, bk_sb, kt, "ktile"),
                    ):
                        wjt = ldp.tile([P, NCH, P], R, tag=f"wj_{tag}",
                                       bufs=2, name="wjt")
                        nc.sync.dma_start(wjt[:, :, :], w_r[:, :, j, :])
                        dtile = qktp.tile([P, S], R, tag=f"{tag}{j}",
                                          name=f"{tag}{j}")
                        dst[j] = dtile
                        for n in range(NHALF):
                            ps = psA.tile([P, HF], F32, tag="ps", name="ps_qk")
                            for c in range(NCH):
                                nc.tensor.matmul(
                                    ps[:, :],
                                    wjt[:, c, :],
                                    hst[c][:, n * HF : (n + 1) * HF],
                                    start=(c == 0),
                                    stop=(c == NCH - 1),
                                )
                            nc.scalar.activation(
                                dtile[:, n * HF : (n + 1) * HF],
                                ps[:, :],
                                AF.Identity,
                                bias=b_sb[:, j : j + 1],
                                scale=1.0,
                            )

            # --- attention, software-pipelined over heads ---
            # scores+exp of head h+1 are emitted before the ctx/normalize
            # work of head h so the tensor engine never sits idle waiting
            # for ScalarE exp / VectorE normalize (HAM stays warm).
            with ExitStack() as att:
                attp = att.enter_context(tc.tile_pool(name="attp", bufs=1))

                def emit_scores(h):
                    t, r = h // 2, (h % 2) * HD
                    exp_tiles = []
                    for c in range(NCH):
                        ps = ps_sc.tile([P, S], F32, tag="sc", name="ps_sc")
                        for n in range(NHALF):
                            nc.tensor.matmul(
                                ps[:, n * HF : (n + 1) * HF],
                                kt[t][r : r + HD, c * P : (c + 1) * P],
                                qt[t][r : r + HD, n * HF : (n + 1) * HF],
                                start=True,
                                stop=True,
                            )
                        e = attp.tile([P, S], R, tag="expt", bufs=16, name="expt")
                        nc.scalar.activation(e[:, :], ps[:, :], AF.Exp, scale=0.125)
                        exp_tiles.append(e)
                    return exp_tiles

                def emit_tail(h, exp_tiles):
                    bc_sb = attp.tile([P, S], F32, tag="bc", bufs=3, name="bc_sb")
                    for n in range(NHALF):
                        psc = ps_ctx.tile([HD + 1, HF], F32, tag="ctx", name="ps_ctx")
                        for c in range(NCH):
                            nc.tensor.matmul(
                                psc[:, :],
                                vt[c][:, h, :],
                                exp_tiles[c][:, n * HF : (n + 1) * HF],
                                start=(c == 0),
                                stop=(c == NCH - 1),
                            )
                        recip = attp.tile([HD + 1, HF], R, tag="recip", bufs=2,
                                          name="recip")
                        nc.vector.reciprocal(recip[HD : HD + 1, :], psc[HD : HD + 1, :])
                        psb = ps_bc.tile([P, HF], F32, tag="bcp", name="ps_bc")
                        nc.tensor.matmul(
                            psb[:, :],
                            ones_r[HD : HD + 1, :],
                            recip[HD : HD + 1, :],
                            start=True,
                            stop=True,
                        )
                        nc.scalar.copy(bc_sb[:, n * HF : (n + 1) * HF], psb[:, :])
                        # normalized ctx^T slice for this head -> DRAM
                        ctx_sb = attp.tile([HD, HF], R, tag="ctxsb", bufs=3,
                                           name="ctx_sb")
                        nc.vector.tensor_mul(
                            ctx_sb[:, :],
                            psc[0:HD, :],
                            bc_sb[0:HD, n * HF : (n + 1) * HF],
                        )
                        nc.sync.dma_start(
                            ctx_dram[h * HD : (h + 1) * HD, n * HF : (n + 1) * HF],
                            ctx_sb[:, :],
                        )
                    for c in range(NCH):
                        pr = attp.tile([P, S], F32, tag="probs", bufs=4, name="pr")
                        nc.vector.tensor_mul(pr[:, :], exp_tiles[c].bitcast(F32)[:, :],
                                             bc_sb[:, :])
                        nc.sync.dma_start(probsT[h, c * P : (c + 1) * P, :], pr[:, :])

                prev = emit_scores(0)
                for h in range(1, H):
                    cur = emit_scores(h)
                    emit_tail(h - 1, prev)
                    prev = cur
                emit_tail(H - 1, prev)

        # --- output projection: out[s, do] = ctx @ Wo + bo ---
        with ExitStack() as fin:
            outp = fin.enter_context(tc.tile_pool(name="outp", bufs=1))
            wot = [outp.tile([P, D], R, tag=f"wo{g}", name=f"wo{g}")
                   for g in range(NCH)]
            ctxp = [outp.tile([P, S], R, tag=f"ctxp{g}", name=f"ctxp{g}")
                    for g in range(NCH)]
            for g in range(NCH):
                nc.sync.dma_start(wot[g][:, :], wo[g * P : (g + 1) * P, :].bitcast(R))
                nc.sync.dma_start(ctxp[g][:, :], ctx_dram[g * P : (g + 1) * P, :])
            for i in range(NCH):
                o_sb = outp.tile([P, D], F32, tag="osb", bufs=3, name="o_sb")
                for n in range(NHALF):
                    ps = psA.tile([P, HF], F32, tag="ps", name="ps_out")
                    for g in range(NCH):
                        nc.tensor.matmul(
                            ps[:, :],
                            ctxp[g][:, i * P : (i + 1) * P],
                            wot[g][:, n * HF : (n + 1) * HF],
                            start=(g == 0),
                            stop=False,
                        )
                    nc.tensor.matmul(
                        ps[:, :],
                        ones_r[0:1, :],
                        bo_row[0:1, n * HF : (n + 1) * HF],
                        start=False,
                        stop=True,
                    )
                    nc.scalar.copy(o_sb[:, n * HF : (n + 1) * HF], ps[:, :])
                nc.sync.dma_start(out[i * P : (i + 1) * P, :], o_sb[:, :])


def build_program():
    nc = bacc.Bacc("TRN2", target_bir_lowering=False, debug=False)
    io = {
        "hsT": nc.dram_tensor("hsT", [D, S], F32, kind="ExternalInput").ap(),
        "Wq": nc.dram_tensor("Wq", [D, D], F32, kind="ExternalInput").ap(),
        "bq": nc.dram_tensor("bq", [D], F32, kind="ExternalInput").ap(),
        "Wk": nc.dram_tensor("Wk", [D, D], F32, kind="ExternalInput").ap(),
        "bk": nc.dram_tensor("bk", [D], F32, kind="ExternalInput").ap(),
        "Wv": nc.dram_tensor("Wv", [D, D], F32, kind="ExternalInput").ap(),
        "bv": nc.dram_tensor("bv", [D], F32, kind="ExternalInput").ap(),
        "Wo": nc.dram_tensor("Wo", [D, D], F32, kind="ExternalInput").ap(),
        "bo": nc.dram_tensor("bo", [D], F32, kind="ExternalInput").ap(),
        "out": nc.dram_tensor("out", [S, D], F32, kind="ExternalOutput").ap(),
        "probsT": nc.dram_tensor("probsT", [H, S, S], F32, kind="ExternalOutput").ap(),
    }
    with nc.allow_low_precision(reason="fp32r matmul input rounding"), \
            tile.TileContext(nc) as tc:
        _emit(tc, io)
    nc.compile()
    return nc


_program = None
last_exec_time_ns = None
last_mean_exec_time_ns = None


def _get_program():
    global _program
    if _program is None:
        _program = build_program()
    return _program


def kernel(hidden_states, Wq, bq, Wk, bk, Wv, bv, Wo, bo):
    """Full-input, full-output attention. Shards batch over 8 cores."""
    global last_exec_time_ns, last_mean_exec_time_ns
    hs = np.asarray(hidden_states, dtype=np.float32)
    shared = {
        "Wq": np.ascontiguousarray(np.asarray(Wq, np.float32)),
        "bq": np.ascontiguousarray(np.asarray(bq, np.float32)),
        "Wk": np.ascontiguousarray(np.asarray(Wk, np.float32)),
        "bk": np.ascontiguousarray(np.asarray(bk, np.float32)),
        "Wv": np.ascontiguousarray(np.asarray(Wv, np.float32)),
        "bv": np.ascontiguousarray(np.asarray(bv, np.float32)),
        "Wo": np.ascontiguousarray(np.asarray(Wo, np.float32)),
        "bo": np.ascontiguousarray(np.asarray(bo, np.float32)),
    }
    in_maps = [
        {"hsT": np.ascontiguousarray(hs[b].T), **shared} for b in range(NCORES)
    ]
    nc = _get_program()
    trace = os.environ.get("KERNEL_TRACE", "0") == "1"
    res = run_bass_kernel_spmd(nc, in_maps, core_ids=list(range(NCORES)),
                               trace=trace)
    last_exec_time_ns = res.exec_time_ns
    last_mean_exec_time_ns = res.mean_exec_time_ns

    out = np.empty((B, S, D), np.float32)
    probs = np.empty((B, H, S, S), np.float32)
    for b in range(NCORES):
        out[b] = res.results[b]["out"]
        probs[b] = res.results[b]["probsT"].transpose(0, 2, 1)
    return out, probs


# revision 13
# speedup vs baseline: 1.0904x; 1.0904x over previous
"""Multi-head attention Bass/Tile kernel for Trainium2, 8-core SPMD.

Problem: B=8, S=1024, D=1024, H=16 (head dim 64) attention that returns
both the attention output [B,S,D] and the softmax probabilities
[B,H,S,S].

Sharding: data-parallel over batch -- core b computes batch element b.

Per-core design (one batch element):
  - The host passes hs[b].T (``hsT`` [D,S]) so the Q/K projections can
    produce QT/KT in transposed [d, s] layout directly (the PE contracts
    over the partition dim of both operands, so hs always appears
    transposed; transposing on the host is free).
  - scores are computed transposed: scoresT[sk, sq] = K Q^T via
    lhsT=KT_h, rhs=QT_h (contraction over the 64-dim head axis).
  - exp on the scalar engine with the 1/sqrt(64) scale folded in.
  - V is kept in natural [sk, dv] layout with a ones column appended per
    head, so each context matmul (lhsT=V_h|1, rhs=expT_h) also produces
    the softmax denominators in psum row 64.
  - probs output is written to DRAM as [H, sk, sq] (contiguous stores);
    the host transposes to [H, sq, sk] while gathering (f32 DMA
    transpose does not exist on TRN2, and recomputing scores in the
    other orientation would double the scalar-engine exp work).
  - normalized ctx^T is staged through a DRAM scratch and re-read
    pair-packed so the out-projection runs K=128 matmuls and produces
    the attention output in natural [s, d] layout directly.
  - All matmuls run as float32r (full PE rate at free-dim >= 256).
    Walrus requires fp32r matmul operands to be *produced* rounded, so
    every matmul-feeding tile is dtype float32r (DMA loads bitcast the
    f32 DRAM side; ACT/DVE producers write f32r directly).
"""

import os

os.environ.setdefault("MYCRO_LOCAL_CACHE", "1")

from contextlib import ExitStack

import numpy as np

import concourse.bass as bass  # noqa: F401  (bass must import before tile)
import concourse.mybir as mybir
import concourse.tile as tile
from concourse import bacc
from concourse.bass_utils import run_bass_kernel_spmd

F32 = mybir.dt.float32
F32R = mybir.dt.float32r
AF = mybir.ActivationFunctionType

B, S, D, H, HD = 8, 1024, 1024, 16, 64
P = 128
NCH = D // P  # 8 chunks of 128 along d or s
HF = 512  # fp32 psum bank free-dim limit
NHALF = S // HF  # 2
NCORES = 8


def _emit(tc, io):
    nc = tc.nc
    hsT, wq, bq, wk, bk, wv, bv, wo, bo = (
        io["hsT"], io["Wq"], io["bq"], io["Wk"], io["bk"],
        io["Wv"], io["bv"], io["Wo"], io["bo"],
    )
    out, probsT = io["out"], io["probsT"]
    R = F32R

    with ExitStack() as top:
        const = top.enter_context(tc.tile_pool(name="const", bufs=1))
        ones_f = const.tile([P, P], F32, tag="ones_f", name="ones_f")
        nc.vector.memset(ones_f[:, :], 1.0)
        # fp32r view of ones for rank-1 bias / broadcast matmuls
        # (memset cannot write f32r directly; a DVE copy can)
        ones_r = const.tile([P, P], R, tag="ones_r", name="ones_r")
        nc.vector.tensor_copy(ones_r[:, :], ones_f[:, :])
        bq_sb = const.tile([P, NCH], F32, tag="bq_sb", name="bq_sb")
        nc.sync.dma_start(bq_sb[:, :], bq.rearrange("(c p) -> p c", p=P))
        bk_sb = const.tile([P, NCH], F32, tag="bk_sb", name="bk_sb")
        nc.sync.dma_start(bk_sb[:, :], bk.rearrange("(c p) -> p c", p=P))
        bv_row = const.tile([1, D], R, tag="bv_row", name="bv_row")
        nc.sync.dma_start(bv_row[:, :], bv.rearrange("(o d) -> o d", o=1).bitcast(R))
        bo_row = const.tile([1, D], R, tag="bo_row", name="bo_row")
        nc.sync.dma_start(bo_row[:, :], bo.rearrange("(o d) -> o d", o=1).bitcast(R))

        # PSUM pools all at top level: psA 2 + sc 3x2 = 8 banks total
        # (psA doubles as the bc-broadcast bank during attention).
        psA = top.enter_context(tc.tile_pool(name="psA", bufs=2, space="PSUM"))
        ps_sc = top.enter_context(tc.tile_pool(name="ps_sc", bufs=2, space="PSUM"))
        ps_ctx = top.enter_context(tc.tile_pool(name="ps_ctx", bufs=2, space="PSUM"))

        # Normalized ctx^T staged through DRAM ([dc, s]); re-read in the
        # out-projection as pair-packed [128, S] chunks (keeps SBUF small
        # and the out-proj matmuls at K=128).
        ctx_dram = nc.dram_tensor("ctx_scratch", [D, S], R, kind="Internal").ap()

        with ExitStack() as mid:
            qkvp = mid.enter_context(tc.tile_pool(name="qkvp", bufs=1))
            vt = [qkvp.tile([P, H, HD + 1], R, tag=f"v{i}", name=f"v{i}")
                  for i in range(NCH)]
            for i in range(NCH):
                # ones column per head (f32r via DVE copy from f32 ones)
                nc.vector.tensor_copy(
                    vt[i][:, :, HD : HD + 1],
                    ones_f[:, 0:H].rearrange("p (h o) -> p h o", o=1),
                )
            qktp = mid.enter_context(tc.tile_pool(name="qktp", bufs=1))
            qt = [None] * NCH
            kt = [None] * NCH

            with ExitStack() as ld:
                ldp = ld.enter_context(tc.tile_pool(name="ldp", bufs=1))
                hst = []
                for c in range(NCH):
                    t = ldp.tile([P, S], R, tag=f"hst{c}", bufs=1, name=f"hst{c}")
                    nc.sync.dma_start(t[:, :], hsT[c * P : (c + 1) * P, :].bitcast(R))
                    hst.append(t)

                # --- V projection: V[sk, dv] = hs @ Wv + bv ---
                wv_r = wv.rearrange("(c p) d -> p c d", p=P).bitcast(R)
                for n in range(NHALF):
                    wvt = ldp.tile([P, NCH, HF], R, tag="wv", bufs=1, name="wvt")
                    nc.sync.dma_start(wvt[:, :, :], wv_r[:, :, n * HF : (n + 1) * HF])
                    for i in range(NCH):
                        ps = psA.tile([P, HF], F32, tag="ps", name="ps_v")
                        for c in range(NCH):
                            nc.tensor.matmul(
                                ps[:, :],
                                hst[c][:, i * P : (i + 1) * P],
                                wvt[:, c, :],
                                start=(c == 0),
                                stop=False,
                            )
                        # bias as a rank-1 (ones ⊗ bv) accumulation
                        nc.tensor.matmul(
                            ps[:, :],
                            ones_r[0:1, :],
                            bv_row[0:1, n * HF : (n + 1) * HF],
                            start=False,
                            stop=True,
                        )
                        # evict on ScalarE (keeps VectorE free for the
                        # attention-phase normalize work)
                        nc.scalar.copy(
                            vt[i][:, n * 8 : (n + 1) * 8, 0:HD],
                            ps.rearrange("p (h e) -> p h e", e=HD),
                        )

                # --- Q/K projections into transposed [do, s] layout ---
                wq_r = wq.rearrange("(c p) (j q) -> p c j q", p=P, q=P).bitcast(R)
                wk_r = wk.rearrange("(c p) (j q) -> p c j q", p=P, q=P).bitcast(R)
                for j in range(NCH):
                    for (w_r, b_sb, dst, tag) in (
                        (wq_r, bq_sb, qt, "qtile"),
                        (wk_r, bk_sb, kt, "ktile"),
                    ):
                        wjt = ldp.tile([P, NCH, P], R, tag=f"wj_{tag}",
                                       bufs=2, name="wjt")
                        nc.sync.dma_start(wjt[:, :, :], w_r[:, :, j, :])
                        dtile = qktp.tile([P, S], R, tag=f"{tag}{j}",
                                          name=f"{tag}{j}")
                        dst[j] = dtile
                        for n in range(NHALF):
                            ps = psA.tile([P, HF], F32, tag="ps", name="ps_qk")
                            for c in range(NCH):
                                nc.tensor.matmul(
                                    ps[:, :],
                                    wjt[:, c, :],
                                    hst[c][:, n * HF : (n + 1) * HF],
                                    start=(c == 0),
                                    stop=(c == NCH - 1),
                                )
                            nc.scalar.activation(
                                dtile[:, n * HF : (n + 1) * HF],
                                ps[:, :],
                                AF.Identity,
                                bias=b_sb[:, j : j + 1],
                                scale=1.0,
                            )

            # --- attention, software-pipelined over heads ---
            # The scores matmuls of head h+1 are interleaved chunk-by-chunk
            # with the ctx matmuls of head h so the tensor engine's stalls
            # stay short (HAM keeps the PE at full clock), while ScalarE
            # runs exp and VectorE/GpSimd split the probs normalization.
            n_gps = int(os.environ.get("GPSIMD_CHUNKS", "4"))
            with ExitStack() as att:
                attp = att.enter_context(tc.tile_pool(name="attp", bufs=1))

                def emit_scores_chunk(h, c):
                    t, r = h // 2, (h % 2) * HD
                    ps = ps_sc.tile([P, S], F32, tag="sc", name="ps_sc")
                    for n in range(NHALF):
                        nc.tensor.matmul(
                            ps[:, n * HF : (n + 1) * HF],
                            kt[t][r : r + HD, c * P : (c + 1) * P],
                            qt[t][r : r + HD, n * HF : (n + 1) * HF],
                            start=True,
                            stop=True,
                        )
                    e = attp.tile([P, S], R, tag="expt", bufs=16, name="expt")
                    nc.scalar.activation(e[:, :], ps[:, :], AF.Exp, scale=0.125)
                    return e

                def emit_head(h, exp_tiles, nxt):
                    """ctx+normalize for head h, interleaving the scores
                    matmuls of head ``nxt`` between ctx chunks."""
                    nxt_tiles = []
                    pscs = [ps_ctx.tile([HD + 1, HF], F32, tag="ctx",
                                        name="ps_ctx") for _ in range(NHALF)]
                    for c in range(NCH):
                        if nxt is not None:
                            nxt_tiles.append(emit_scores_chunk(nxt, c))
                        for n in range(NHALF):
                            nc.tensor.matmul(
                                pscs[n][:, :],
                                vt[c][:, h, :],
                                exp_tiles[c][:, n * HF : (n + 1) * HF],
                                start=(c == 0),
                                stop=(c == NCH - 1),
                            )
                    bc_sb = attp.tile([P, S], F32, tag="bc", bufs=2, name="bc_sb")
                    for n in range(NHALF):
                        psc = pscs[n]
                        # 1/sums on ScalarE: recip = exp(-ln(sums)); ln and
                        # exp share one activation table set.  The DVE
                        # RECIPROCAL (iterative divide) costs ~3.4us per
                        # row and was the single most expensive DVE op.
                        lns = attp.tile([HD + 1, HF], F32, tag="lns", bufs=2,
                                        name="lns")
                        nc.scalar.activation(lns[HD : HD + 1, :],
                                             psc[HD : HD + 1, :], AF.Ln)
                        recip = attp.tile([HD + 1, HF], R, tag="recip", bufs=2,
                                          name="recip")
                        nc.scalar.activation(recip[HD : HD + 1, :],
                                             lns[HD : HD + 1, :], AF.Exp,
                                             scale=-1.0)
                        psb = psA.tile([P, HF], F32, tag="ps", name="ps_bc")
                        nc.tensor.matmul(
                            psb[:, :],
                            ones_r[HD : HD + 1, :],
                            recip[HD : HD + 1, :],
                            start=True,
                            stop=True,
                        )
                        nc.vector.tensor_copy(bc_sb[:, n * HF : (n + 1) * HF],
                                              psb[:, :])
                        # normalized ctx^T slice for this head -> DRAM
                        ctx_sb = attp.tile([HD, HF], R, tag="ctxsb", bufs=2,
                                           name="ctx_sb")
                        nc.vector.tensor_mul(
                            ctx_sb[:, :],
                            psc[0:HD, :],
                            bc_sb[0:HD, n * HF : (n + 1) * HF],
                        )
                        nc.sync.dma_start(
                            ctx_dram[h * HD : (h + 1) * HD, n * HF : (n + 1) * HF],
                            ctx_sb[:, :],
                        )
                    for c in range(NCH):
                        pr = attp.tile([P, S], F32, tag="probs", bufs=4, name="pr")
                        eng = nc.gpsimd if c < n_gps else nc.vector
                        eng.tensor_mul(pr[:, :], exp_tiles[c].bitcast(F32)[:, :],
                                       bc_sb[:, :])
                        nc.sync.dma_start(probsT[h, c * P : (c + 1) * P, :], pr[:, :])
                    return nxt_tiles

                cur = [emit_scores_chunk(0, c) for c in range(NCH)]
                for h in range(H):
                    cur = emit_head(h, cur, h + 1 if h + 1 < H else None)

        # --- output projection: out[s, do] = ctx @ Wo + bo ---
        with ExitStack() as fin:
            outp = fin.enter_context(tc.tile_pool(name="outp", bufs=1))
            wot = [outp.tile([P, D], R, tag=f"wo{g}", name=f"wo{g}")
                   for g in range(NCH)]
            ctxp = [outp.tile([P, S], R, tag=f"ctxp{g}", name=f"ctxp{g}")
                    for g in range(NCH)]
            for g in range(NCH):
                nc.sync.dma_start(wot[g][:, :], wo[g * P : (g + 1) * P, :].bitcast(R))
                nc.sync.dma_start(ctxp[g][:, :], ctx_dram[g * P : (g + 1) * P, :])
            for i in range(NCH):
                o_sb = outp.tile([P, D], F32, tag="osb", bufs=3, name="o_sb")
                for n in range(NHALF):
                    ps = psA.tile([P, HF], F32, tag="ps", name="ps_out")
                    for g in range(NCH):
                        nc.tensor.matmul(
                            ps[:, :],
                            ctxp[g][:, i * P : (i + 1) * P],
                            wot[g][:, n * HF : (n + 1) * HF],
                            start=(g == 0),
                            stop=False,
                        )
                    nc.tensor.matmul(
                        ps[:, :],
                        ones_r[0:1, :],
                        bo_row[0:1, n * HF : (n + 1) * HF],
                        start=False,
                        stop=True,
                    )
                    nc.scalar.copy(o_sb[:, n * HF : (n + 1) * HF], ps[:, :])
                nc.sync.dma_start(out[i * P : (i + 1) * P, :], o_sb[:, :])


def build_program():
    nc = bacc.Bacc("TRN2", target_bir_lowering=False, debug=False)
    io = {
        "hsT": nc.dram_tensor("hsT", [D, S], F32, kind="ExternalInput").ap(),
        "Wq": nc.dram_tensor("Wq", [D, D], F32, kind="ExternalInput").ap(),
        "bq": nc.dram_tensor("bq", [D], F32, kind="ExternalInput").ap(),
        "Wk": nc.dram_tensor("Wk", [D, D], F32, kind="ExternalInput").ap(),
        "bk": nc.dram_tensor("bk", [D], F32, kind="ExternalInput").ap(),
        "Wv": nc.dram_tensor("Wv", [D, D], F32, kind="ExternalInput").ap(),
        "bv": nc.dram_tensor("bv", [D], F32, kind="ExternalInput").ap(),
        "Wo": nc.dram_tensor("Wo", [D, D], F32, kind="ExternalInput").ap(),
        "bo": nc.dram_tensor("bo", [D], F32, kind="ExternalInput").ap(),
        "out": nc.dram_tensor("out", [S, D], F32, kind="ExternalOutput").ap(),
        "probsT": nc.dram_tensor("probsT", [H, S, S], F32, kind="ExternalOutput").ap(),
    }
    with nc.allow_low_precision(reason="fp32r matmul input rounding"), \
            tile.TileContext(nc) as tc:
        _emit(tc, io)
    nc.compile()
    return nc


_program = None
last_exec_time_ns = None
last_mean_exec_time_ns = None


def _get_program():
    global _program
    if _program is None:
        _program = build_program()
    return _program


def kernel(hidden_states, Wq, bq, Wk, bk, Wv, bv, Wo, bo):
    """Full-input, full-output attention. Shards batch over 8 cores."""
    global last_exec_time_ns, last_mean_exec_time_ns
    hs = np.asarray(hidden_states, dtype=np.float32)
    shared = {
        "Wq": np.ascontiguousarray(np.asarray(Wq, np.float32)),
        "bq": np.ascontiguousarray(np.asarray(bq, np.float32)),
        "Wk": np.ascontiguousarray(np.asarray(Wk, np.float32)),
        "bk": np.ascontiguousarray(np.asarray(bk, np.float32)),
        "Wv": np.ascontiguousarray(np.asarray(Wv, np.float32)),
        "bv": np.ascontiguousarray(np.asarray(bv, np.float32)),
        "Wo": np.ascontiguousarray(np.asarray(Wo, np.float32)),
        "bo": np.ascontiguousarray(np.asarray(bo, np.float32)),
    }
    in_maps = [
        {"hsT": np.ascontiguousarray(hs[b].T), **shared} for b in range(NCORES)
    ]
    nc = _get_program()
    trace = os.environ.get("KERNEL_TRACE", "0") == "1"
    res = run_bass_kernel_spmd(nc, in_maps, core_ids=list(range(NCORES)),
                               trace=trace)
    last_exec_time_ns = res.exec_time_ns
    last_mean_exec_time_ns = res.mean_exec_time_ns

    out = np.empty((B, S, D), np.float32)
    probs = np.empty((B, H, S, S), np.float32)
    for b in range(NCORES):
        out[b] = res.results[b]["out"]
        probs[b] = res.results[b]["probsT"].transpose(0, 2, 1)
    return out, probs


# revision 15
# speedup vs baseline: 1.1113x; 1.0192x over previous
"""Multi-head attention Bass/Tile kernel for Trainium2, 8-core SPMD.

Problem: B=8, S=1024, D=1024, H=16 (head dim 64) attention that returns
both the attention output [B,S,D] and the softmax probabilities
[B,H,S,S].

Sharding: data-parallel over batch -- core b computes batch element b.

Per-core design (one batch element):
  - The host passes hs[b].T (``hsT`` [D,S]) so the Q/K projections can
    produce QT/KT in transposed [d, s] layout directly (the PE contracts
    over the partition dim of both operands, so hs always appears
    transposed; transposing on the host is free).
  - scores are computed transposed: scoresT[sk, sq] = K Q^T via
    lhsT=KT_h, rhs=QT_h (contraction over the 64-dim head axis).
  - exp on the scalar engine with the 1/sqrt(64) scale folded in.
  - V is kept in natural [sk, dv] layout with a ones column appended per
    head, so each context matmul (lhsT=V_h|1, rhs=expT_h) also produces
    the softmax denominators in psum row 64.
  - probs output is written to DRAM as [H, sk, sq] (contiguous stores);
    the host transposes to [H, sq, sk] while gathering (f32 DMA
    transpose does not exist on TRN2, and recomputing scores in the
    other orientation would double the scalar-engine exp work).
  - normalized ctx^T is staged through a DRAM scratch and re-read
    pair-packed so the out-projection runs K=128 matmuls and produces
    the attention output in natural [s, d] layout directly.
  - All matmuls run as float32r (full PE rate at free-dim >= 256).
    Walrus requires fp32r matmul operands to be *produced* rounded, so
    every matmul-feeding tile is dtype float32r (DMA loads bitcast the
    f32 DRAM side; ACT/DVE producers write f32r directly).
"""

import os

os.environ.setdefault("MYCRO_LOCAL_CACHE", "1")

from contextlib import ExitStack

import numpy as np

import concourse.bass as bass  # noqa: F401  (bass must import before tile)
import concourse.mybir as mybir
import concourse.tile as tile
from concourse import bacc
from concourse.bass_utils import run_bass_kernel_spmd

F32 = mybir.dt.float32
F32R = mybir.dt.float32r
AF = mybir.ActivationFunctionType

B, S, D, H, HD = 8, 1024, 1024, 16, 64
P = 128
NCH = D // P  # 8 chunks of 128 along d or s
HF = 512  # fp32 psum bank free-dim limit
NHALF = S // HF  # 2
NCORES = 8


def _emit(tc, io):
    nc = tc.nc
    hsT, wq, bq, wk, bk, wv, bv, wo, bo = (
        io["hsT"], io["Wq"], io["bq"], io["Wk"], io["bk"],
        io["Wv"], io["bv"], io["Wo"], io["bo"],
    )
    out, probsT = io["out"], io["probsT"]
    R = F32R

    with ExitStack() as top:
        const = top.enter_context(tc.tile_pool(name="const", bufs=1))
        ones_f = const.tile([P, P], F32, tag="ones_f", name="ones_f")
        nc.vector.memset(ones_f[:, :], 1.0)
        # fp32r view of ones for rank-1 bias / broadcast matmuls
        # (memset cannot write f32r directly; a DVE copy can)
        ones_r = const.tile([P, P], R, tag="ones_r", name="ones_r")
        nc.vector.tensor_copy(ones_r[:, :], ones_f[:, :])
        bq_sb = const.tile([P, NCH], F32, tag="bq_sb", name="bq_sb")
        nc.sync.dma_start(bq_sb[:, :], bq.rearrange("(c p) -> p c", p=P))
        bk_sb = const.tile([P, NCH], F32, tag="bk_sb", name="bk_sb")
        nc.sync.dma_start(bk_sb[:, :], bk.rearrange("(c p) -> p c", p=P))
        bv_row = const.tile([1, D], R, tag="bv_row", name="bv_row")
        nc.sync.dma_start(bv_row[:, :], bv.rearrange("(o d) -> o d", o=1).bitcast(R))
        bo_row = const.tile([1, D], R, tag="bo_row", name="bo_row")
        nc.sync.dma_start(bo_row[:, :], bo.rearrange("(o d) -> o d", o=1).bitcast(R))

        # PSUM pools all at top level: psA 2 + sc 3x2 = 8 banks total
        # (psA doubles as the bc-broadcast bank during attention).
        psA = top.enter_context(tc.tile_pool(name="psA", bufs=2, space="PSUM"))
        ps_sc = top.enter_context(tc.tile_pool(name="ps_sc", bufs=2, space="PSUM"))
        ps_ctx = top.enter_context(tc.tile_pool(name="ps_ctx", bufs=2, space="PSUM"))

        # Normalized ctx^T staged through DRAM ([dc, s]); re-read in the
        # out-projection as pair-packed [128, S] chunks (keeps SBUF small
        # and the out-proj matmuls at K=128).
        ctx_dram = nc.dram_tensor("ctx_scratch", [D, S], R, kind="Internal").ap()

        with ExitStack() as mid:
            qkvp = mid.enter_context(tc.tile_pool(name="qkvp", bufs=1))
            vt = [qkvp.tile([P, H, HD + 1], R, tag=f"v{i}", name=f"v{i}")
                  for i in range(NCH)]
            for i in range(NCH):
                # ones column per head (f32r via DVE copy from f32 ones)
                nc.vector.tensor_copy(
                    vt[i][:, :, HD : HD + 1],
                    ones_f[:, 0:H].rearrange("p (h o) -> p h o", o=1),
                )
            qktp = mid.enter_context(tc.tile_pool(name="qktp", bufs=1))
            qt = [None] * NCH
            kt = [None] * NCH

            with ExitStack() as ld:
                ldp = ld.enter_context(tc.tile_pool(name="ldp", bufs=1))
                hst = []
                for c in range(NCH):
                    t = ldp.tile([P, S], R, tag=f"hst{c}", bufs=1, name=f"hst{c}")
                    nc.sync.dma_start(t[:, :], hsT[c * P : (c + 1) * P, :].bitcast(R))
                    hst.append(t)

                # --- V projection: V[sk, dv] = hs @ Wv + bv ---
                wv_r = wv.rearrange("(c p) d -> p c d", p=P).bitcast(R)
                for n in range(NHALF):
                    wvt = ldp.tile([P, NCH, HF], R, tag="wv", bufs=1, name="wvt")
                    nc.sync.dma_start(wvt[:, :, :], wv_r[:, :, n * HF : (n + 1) * HF])
                    for i in range(NCH):
                        ps = psA.tile([P, HF], F32, tag="ps", name="ps_v")
                        for c in range(NCH):
                            nc.tensor.matmul(
                                ps[:, :],
                                hst[c][:, i * P : (i + 1) * P],
                                wvt[:, c, :],
                                start=(c == 0),
                                stop=False,
                            )
                        # bias as a rank-1 (ones ⊗ bv) accumulation
                        nc.tensor.matmul(
                            ps[:, :],
                            ones_r[0:1, :],
                            bv_row[0:1, n * HF : (n + 1) * HF],
                            start=False,
                            stop=True,
                        )
                        # evict on ScalarE (keeps VectorE free for the
                        # attention-phase normalize work)
                        nc.scalar.copy(
                            vt[i][:, n * 8 : (n + 1) * 8, 0:HD],
                            ps.rearrange("p (h e) -> p h e", e=HD),
                        )

                # --- Q/K projections into transposed [do, s] layout ---
                wq_r = wq.rearrange("(c p) (j q) -> p c j q", p=P, q=P).bitcast(R)
                wk_r = wk.rearrange("(c p) (j q) -> p c j q", p=P, q=P).bitcast(R)
                for j in range(NCH):
                    for (w_r, b_sb, dst, tag) in (
                        (wq_r, bq_sb, qt, "qtile"),
                        (wk_r, bk_sb, kt, "ktile"),
                    ):
                        wjt = ldp.tile([P, NCH, P], R, tag=f"wj_{tag}",
                                       bufs=2, name="wjt")
                        nc.sync.dma_start(wjt[:, :, :], w_r[:, :, j, :])
                        dtile = qktp.tile([P, S], R, tag=f"{tag}{j}",
                                          name=f"{tag}{j}")
                        dst[j] = dtile
                        for n in range(NHALF):
                            ps = psA.tile([P, HF], F32, tag="ps", name="ps_qk")
                            for c in range(NCH):
                                nc.tensor.matmul(
                                    ps[:, :],
                                    wjt[:, c, :],
                                    hst[c][:, n * HF : (n + 1) * HF],
                                    start=(c == 0),
                                    stop=(c == NCH - 1),
                                )
                            nc.scalar.activation(
                                dtile[:, n * HF : (n + 1) * HF],
                                ps[:, :],
                                AF.Identity,
                                bias=b_sb[:, j : j + 1],
                                scale=1.0,
                            )

            # --- attention, software-pipelined over heads ---
            # The scores matmuls of head h+1 are interleaved chunk-by-chunk
            # with the ctx matmuls of head h so the tensor engine's stalls
            # stay short (HAM keeps the PE at full clock), while ScalarE
            # runs exp and VectorE/GpSimd split the probs normalization.
            n_gps = int(os.environ.get("GPSIMD_CHUNKS", "3"))
            with ExitStack() as att:
                attp = att.enter_context(tc.tile_pool(name="attp", bufs=1))

                def emit_scores_chunk(h, c):
                    t, r = h // 2, (h % 2) * HD
                    ps = ps_sc.tile([P, S], F32, tag="sc", name="ps_sc")
                    for n in range(NHALF):
                        nc.tensor.matmul(
                            ps[:, n * HF : (n + 1) * HF],
                            kt[t][r : r + HD, c * P : (c + 1) * P],
                            qt[t][r : r + HD, n * HF : (n + 1) * HF],
                            start=True,
                            stop=True,
                        )
                    e = attp.tile([P, S], R, tag="expt", bufs=12, name="expt")
                    nc.scalar.activation(e[:, :], ps[:, :], AF.Exp, scale=0.125)
                    return e

                def emit_head(h, exp_tiles, nxt):
                    """ctx+normalize for head h, interleaving the scores
                    matmuls of head ``nxt`` between ctx chunks."""
                    nxt_tiles = []
                    pscs = [ps_ctx.tile([HD + 1, HF], F32, tag="ctx",
                                        name="ps_ctx") for _ in range(NHALF)]
                    for c in range(NCH):
                        if nxt is not None:
                            nxt_tiles.append(emit_scores_chunk(nxt, c))
                        for n in range(NHALF):
                            nc.tensor.matmul(
                                pscs[n][:, :],
                                vt[c][:, h, :],
                                exp_tiles[c][:, n * HF : (n + 1) * HF],
                                start=(c == 0),
                                stop=(c == NCH - 1),
                            )
                    bc_sb = attp.tile([P, S], F32, tag="bc", bufs=2, name="bc_sb")
                    for n in range(NHALF):
                        psc = pscs[n]
                        # 1/sums via the single-op Newton-Raphson DVE
                        # reciprocal (~51 ULP -- far below the fp32r matmul
                        # noise floor).  The exact DVE RECIPROCAL costs
                        # ~3.4us per row (iterative divide), and an ACT
                        # ln/exp pair thrashes the activation table set.
                        recip = attp.tile([HD + 1, HF], F32, tag="recip", bufs=2,
                                          name="recip")
                        # custom-DVE ops misbehave at a non-zero base
                        # partition on HW (uop indexing bug; verified:
                        # base-64 returns -sin(x) garbage).  Run over all
                        # 65 rows from base 0; only row 64 (the sums row)
                        # is consumed.
                        nc.vector.reciprocal_approx_fast(
                            recip[0 : HD + 1, :], psc[0 : HD + 1, :])
                        recip_r = attp.tile([HD + 1, HF], R, tag="recipr",
                                            bufs=2, name="recip_r")
                        nc.vector.tensor_copy(recip_r[HD : HD + 1, :],
                                              recip[HD : HD + 1, :])
                        psb = psA.tile([P, HF], F32, tag="ps", name="ps_bc")
                        nc.tensor.matmul(
                            psb[:, :],
                            ones_r[HD : HD + 1, :],
                            recip_r[HD : HD + 1, :],
                            start=True,
                            stop=True,
                        )
                        nc.vector.tensor_copy(bc_sb[:, n * HF : (n + 1) * HF],
                                              psb[:, :])
                        # normalized ctx^T slice for this head -> DRAM
                        ctx_sb = attp.tile([HD, HF], R, tag="ctxsb", bufs=2,
                                           name="ctx_sb")
                        nc.vector.tensor_mul(
                            ctx_sb[:, :],
                            psc[0:HD, :],
                            bc_sb[0:HD, n * HF : (n + 1) * HF],
                        )
                        nc.sync.dma_start(
                            ctx_dram[h * HD : (h + 1) * HD, n * HF : (n + 1) * HF],
                            ctx_sb[:, :],
                        )
                    for c in range(NCH):
                        pr = attp.tile([P, S], F32, tag="probs", bufs=3, name="pr")
                        eng = nc.gpsimd if c < n_gps else nc.vector
                        eng.tensor_mul(pr[:, :], exp_tiles[c].bitcast(F32)[:, :],
                                       bc_sb[:, :])
                        nc.sync.dma_start(probsT[h, c * P : (c + 1) * P, :], pr[:, :])
                    return nxt_tiles

                cur = [emit_scores_chunk(0, c) for c in range(NCH)]
                for h in range(H):
                    cur = emit_head(h, cur, h + 1 if h + 1 < H else None)

        # --- output projection: out[s, do] = ctx @ Wo + bo ---
        with ExitStack() as fin:
            outp = fin.enter_context(tc.tile_pool(name="outp", bufs=1))
            wot = [outp.tile([P, D], R, tag=f"wo{g}", name=f"wo{g}")
                   for g in range(NCH)]
            ctxp = [outp.tile([P, S], R, tag=f"ctxp{g}", name=f"ctxp{g}")
                    for g in range(NCH)]
            for g in range(NCH):
                nc.sync.dma_start(wot[g][:, :], wo[g * P : (g + 1) * P, :].bitcast(R))
                nc.sync.dma_start(ctxp[g][:, :], ctx_dram[g * P : (g + 1) * P, :])
            for i in range(NCH):
                o_sb = outp.tile([P, D], F32, tag="osb", bufs=3, name="o_sb")
                for n in range(NHALF):
                    ps = ps_sc.tile([P, HF], F32, tag="sc", name="ps_out")
                    for g in range(NCH):
                        nc.tensor.matmul(
                            ps[:, :],
                            ctxp[g][:, i * P : (i + 1) * P],
                            wot[g][:, n * HF : (n + 1) * HF],
                            start=(g == 0),
                            stop=False,
                        )
                    nc.tensor.matmul(
                        ps[:, :],
                        ones_r[0:1, :],
                        bo_row[0:1, n * HF : (n + 1) * HF],
                        start=False,
                        stop=True,
                    )
                    nc.scalar.copy(o_sb[:, n * HF : (n + 1) * HF], ps[:, :])
                nc.sync.dma_start(out[i * P : (i + 1) * P, :], o_sb[:, :])


def build_program():
    nc = bacc.Bacc("TRN2", target_bir_lowering=False, debug=False)
    io = {
        "hsT": nc.dram_tensor("hsT", [D, S], F32, kind="ExternalInput").ap(),
        "Wq": nc.dram_tensor("Wq", [D, D], F32, kind="ExternalInput").ap(),
        "bq": nc.dram_tensor("bq", [D], F32, kind="ExternalInput").ap(),
        "Wk": nc.dram_tensor("Wk", [D, D], F32, kind="ExternalInput").ap(),
        "bk": nc.dram_tensor("bk", [D], F32, kind="ExternalInput").ap(),
        "Wv": nc.dram_tensor("Wv", [D, D], F32, kind="ExternalInput").ap(),
        "bv": nc.dram_tensor("bv", [D], F32, kind="ExternalInput").ap(),
        "Wo": nc.dram_tensor("Wo", [D, D], F32, kind="ExternalInput").ap(),
        "bo": nc.dram_tensor("bo", [D], F32, kind="ExternalInput").ap(),
        "out": nc.dram_tensor("out", [S, D], F32, kind="ExternalOutput").ap(),
        "probsT": nc.dram_tensor("probsT", [H, S, S], F32, kind="ExternalOutput").ap(),
    }
    with nc.allow_low_precision(reason="fp32r matmul input rounding"), \
            tile.TileContext(nc) as tc:
        _emit(tc, io)
    nc.compile()
    return nc


_program = None
last_exec_time_ns = None
last_mean_exec_time_ns = None


def _get_program():
    global _program
    if _program is None:
        _program = build_program()
    return _program


def kernel(hidden_states, Wq, bq, Wk, bk, Wv, bv, Wo, bo):
    """Full-input, full-output attention. Shards batch over 8 cores."""
    global last_exec_time_ns, last_mean_exec_time_ns
    hs = np.asarray(hidden_states, dtype=np.float32)
    shared = {
        "Wq": np.ascontiguousarray(np.asarray(Wq, np.float32)),
        "bq": np.ascontiguousarray(np.asarray(bq, np.float32)),
        "Wk": np.ascontiguousarray(np.asarray(Wk, np.float32)),
        "bk": np.ascontiguousarray(np.asarray(bk, np.float32)),
        "Wv": np.ascontiguousarray(np.asarray(Wv, np.float32)),
        "bv": np.ascontiguousarray(np.asarray(bv, np.float32)),
        "Wo": np.ascontiguousarray(np.asarray(Wo, np.float32)),
        "bo": np.ascontiguousarray(np.asarray(bo, np.float32)),
    }
    in_maps = [
        {"hsT": np.ascontiguousarray(hs[b].T), **shared} for b in range(NCORES)
    ]
    nc = _get_program()
    trace = os.environ.get("KERNEL_TRACE", "0") == "1"
    res = run_bass_kernel_spmd(nc, in_maps, core_ids=list(range(NCORES)),
                               trace=trace)
    last_exec_time_ns = res.exec_time_ns
    last_mean_exec_time_ns = res.mean_exec_time_ns

    out = np.empty((B, S, D), np.float32)
    probs = np.empty((B, H, S, S), np.float32)
    for b in range(NCORES):
        out[b] = res.results[b]["out"]
        probs[b] = res.results[b]["probsT"].transpose(0, 2, 1)
    return out, probs


# revision 16
# speedup vs baseline: 1.1527x; 1.0372x over previous
"""Multi-head attention Bass/Tile kernel for Trainium2, 8-core SPMD.

Problem: B=8, S=1024, D=1024, H=16 (head dim 64) attention that returns
both the attention output [B,S,D] and the softmax probabilities
[B,H,S,S].

Sharding: data-parallel over batch -- core b computes batch element b.

Per-core design (one batch element):
  - The host passes hs[b].T (``hsT`` [D,S]) so the Q/K projections can
    produce QT/KT in transposed [d, s] layout directly (the PE contracts
    over the partition dim of both operands, so hs always appears
    transposed; transposing on the host is free).
  - scores are computed transposed: scoresT[sk, sq] = K Q^T via
    lhsT=KT_h, rhs=QT_h (contraction over the 64-dim head axis).
  - exp on the scalar engine with the 1/sqrt(64) scale folded in.
  - V is kept in natural [sk, dv] layout with a ones column appended per
    head, so each context matmul (lhsT=V_h|1, rhs=expT_h) also produces
    the softmax denominators in psum row 64.
  - probs output is written to DRAM as [H, sk, sq] (contiguous stores);
    the host transposes to [H, sq, sk] while gathering (f32 DMA
    transpose does not exist on TRN2, and recomputing scores in the
    other orientation would double the scalar-engine exp work).
  - normalized ctx^T is staged through a DRAM scratch and re-read
    pair-packed so the out-projection runs K=128 matmuls and produces
    the attention output in natural [s, d] layout directly.
  - All matmuls run as float32r (full PE rate at free-dim >= 256).
    Walrus requires fp32r matmul operands to be *produced* rounded, so
    every matmul-feeding tile is dtype float32r (DMA loads bitcast the
    f32 DRAM side; ACT/DVE producers write f32r directly).
"""

import os

os.environ.setdefault("MYCRO_LOCAL_CACHE", "1")

from contextlib import ExitStack

import numpy as np

import concourse.bass as bass  # noqa: F401  (bass must import before tile)
import concourse.mybir as mybir
import concourse.tile as tile
from concourse import bacc
from concourse.bass_utils import run_bass_kernel_spmd

F32 = mybir.dt.float32
F32R = mybir.dt.float32r
AF = mybir.ActivationFunctionType

B, S, D, H, HD = 8, 1024, 1024, 16, 64
P = 128
NCH = D // P  # 8 chunks of 128 along d or s
HF = 512  # fp32 psum bank free-dim limit
NHALF = S // HF  # 2
NCORES = 8


def _emit(tc, io):
    nc = tc.nc
    hsT, wq, bq, wk, bk, wv, bv, wo, bo = (
        io["hsT"], io["Wq"], io["bq"], io["Wk"], io["bk"],
        io["Wv"], io["bv"], io["Wo"], io["bo"],
    )
    out, probsT = io["out"], io["probsT"]
    R = F32R

    with ExitStack() as top:
        const = top.enter_context(tc.tile_pool(name="const", bufs=1))
        ones_f = const.tile([P, P], F32, tag="ones_f", name="ones_f")
        nc.vector.memset(ones_f[:, :], 1.0)
        # fp32r view of ones for rank-1 bias / broadcast matmuls
        # (memset cannot write f32r directly; a DVE copy can)
        ones_r = const.tile([P, P], R, tag="ones_r", name="ones_r")
        nc.vector.tensor_copy(ones_r[:, :], ones_f[:, :])
        bq_sb = const.tile([P, NCH], F32, tag="bq_sb", name="bq_sb")
        nc.sync.dma_start(bq_sb[:, :], bq.rearrange("(c p) -> p c", p=P))
        bk_sb = const.tile([P, NCH], F32, tag="bk_sb", name="bk_sb")
        nc.sync.dma_start(bk_sb[:, :], bk.rearrange("(c p) -> p c", p=P))
        bv_row = const.tile([1, D], R, tag="bv_row", name="bv_row")
        nc.sync.dma_start(bv_row[:, :], bv.rearrange("(o d) -> o d", o=1).bitcast(R))
        bo_row = const.tile([1, D], R, tag="bo_row", name="bo_row")
        nc.sync.dma_start(bo_row[:, :], bo.rearrange("(o d) -> o d", o=1).bitcast(R))

        # PSUM: one 4-buffer [128,512] pool shared by the QKV chains, ctx
        # accumulators, bc broadcasts and the out-projection (deep rotation
        # keeps PE accumulation chains back-to-back so HAM reaches full
        # clock), plus a 2-buffer [128,1024] pool for scores.  4 + 4 = 8
        # banks.
        psA = top.enter_context(tc.tile_pool(name="psA", bufs=4, space="PSUM"))
        ps_sc = top.enter_context(tc.tile_pool(name="ps_sc", bufs=2, space="PSUM"))

        # Normalized ctx^T staged through DRAM ([dc, s]); re-read in the
        # out-projection as pair-packed [128, S] chunks (keeps SBUF small
        # and the out-proj matmuls at K=128).
        ctx_dram = nc.dram_tensor("ctx_scratch", [D, S], R, kind="Internal").ap()

        with ExitStack() as mid:
            qkvp = mid.enter_context(tc.tile_pool(name="qkvp", bufs=1))
            vt = [qkvp.tile([P, H, HD + 1], R, tag=f"v{i}", name=f"v{i}")
                  for i in range(NCH)]
            for i in range(NCH):
                # ones column per head (f32r via DVE copy from f32 ones)
                nc.vector.tensor_copy(
                    vt[i][:, :, HD : HD + 1],
                    ones_f[:, 0:H].rearrange("p (h o) -> p h o", o=1),
                )
            qktp = mid.enter_context(tc.tile_pool(name="qktp", bufs=1))
            qt = [None] * NCH
            kt = [None] * NCH

            with ExitStack() as ld:
                ldp = ld.enter_context(tc.tile_pool(name="ldp", bufs=1))
                hst = []
                for c in range(NCH):
                    t = ldp.tile([P, S], R, tag=f"hst{c}", bufs=1, name=f"hst{c}")
                    nc.sync.dma_start(t[:, :], hsT[c * P : (c + 1) * P, :].bitcast(R))
                    hst.append(t)

                # --- V projection: V[sk, dv] = hs @ Wv + bv ---
                wv_r = wv.rearrange("(c p) d -> p c d", p=P).bitcast(R)
                for n in range(NHALF):
                    wvt = ldp.tile([P, NCH, HF], R, tag="wv", bufs=1, name="wvt")
                    nc.sync.dma_start(wvt[:, :, :], wv_r[:, :, n * HF : (n + 1) * HF])
                    for i in range(NCH):
                        ps = psA.tile([P, HF], F32, tag="ps", name="ps_v")
                        for c in range(NCH):
                            nc.tensor.matmul(
                                ps[:, :],
                                hst[c][:, i * P : (i + 1) * P],
                                wvt[:, c, :],
                                start=(c == 0),
                                stop=False,
                            )
                        # bias as a rank-1 (ones ⊗ bv) accumulation
                        nc.tensor.matmul(
                            ps[:, :],
                            ones_r[0:1, :],
                            bv_row[0:1, n * HF : (n + 1) * HF],
                            start=False,
                            stop=True,
                        )
                        # evict on ScalarE (keeps VectorE free for the
                        # attention-phase normalize work)
                        nc.scalar.copy(
                            vt[i][:, n * 8 : (n + 1) * 8, 0:HD],
                            ps.rearrange("p (h e) -> p h e", e=HD),
                        )

                # --- Q/K projections into transposed [do, s] layout ---
                wq_r = wq.rearrange("(c p) (j q) -> p c j q", p=P, q=P).bitcast(R)
                wk_r = wk.rearrange("(c p) (j q) -> p c j q", p=P, q=P).bitcast(R)
                for j in range(NCH):
                    for (w_r, b_sb, dst, tag) in (
                        (wq_r, bq_sb, qt, "qtile"),
                        (wk_r, bk_sb, kt, "ktile"),
                    ):
                        wjt = ldp.tile([P, NCH, P], R, tag=f"wj_{tag}",
                                       bufs=2, name="wjt")
                        nc.sync.dma_start(wjt[:, :, :], w_r[:, :, j, :])
                        dtile = qktp.tile([P, S], R, tag=f"{tag}{j}",
                                          name=f"{tag}{j}")
                        dst[j] = dtile
                        for n in range(NHALF):
                            ps = psA.tile([P, HF], F32, tag="ps", name="ps_qk")
                            for c in range(NCH):
                                nc.tensor.matmul(
                                    ps[:, :],
                                    wjt[:, c, :],
                                    hst[c][:, n * HF : (n + 1) * HF],
                                    start=(c == 0),
                                    stop=(c == NCH - 1),
                                )
                            nc.scalar.activation(
                                dtile[:, n * HF : (n + 1) * HF],
                                ps[:, :],
                                AF.Identity,
                                bias=b_sb[:, j : j + 1],
                                scale=1.0,
                            )

            # --- attention, software-pipelined over heads ---
            # The scores matmuls of head h+1 are interleaved chunk-by-chunk
            # with the ctx matmuls of head h so the tensor engine's stalls
            # stay short (HAM keeps the PE at full clock), while ScalarE
            # runs exp and VectorE/GpSimd split the probs normalization.
            n_gps = int(os.environ.get("GPSIMD_CHUNKS", "3"))
            with ExitStack() as att:
                attp = att.enter_context(tc.tile_pool(name="attp", bufs=1))

                def emit_scores_chunk(h, c):
                    t, r = h // 2, (h % 2) * HD
                    ps = ps_sc.tile([P, S], F32, tag="sc", name="ps_sc")
                    for n in range(NHALF):
                        nc.tensor.matmul(
                            ps[:, n * HF : (n + 1) * HF],
                            kt[t][r : r + HD, c * P : (c + 1) * P],
                            qt[t][r : r + HD, n * HF : (n + 1) * HF],
                            start=True,
                            stop=True,
                        )
                    e = attp.tile([P, S], R, tag="expt", bufs=12, name="expt")
                    nc.scalar.activation(e[:, :], ps[:, :], AF.Exp, scale=0.125)
                    return e

                def emit_head(h, exp_tiles, nxt):
                    """ctx+normalize for head h, interleaving the scores
                    matmuls of head ``nxt`` between ctx chunks."""
                    nxt_tiles = []
                    pscs = [psA.tile([P, HF], F32, tag="ps",
                                     name="ps_ctx") for _ in range(NHALF)]
                    for c in range(NCH):
                        if nxt is not None:
                            nxt_tiles.append(emit_scores_chunk(nxt, c))
                        for n in range(NHALF):
                            nc.tensor.matmul(
                                pscs[n][0 : HD + 1, :],
                                vt[c][:, h, :],
                                exp_tiles[c][:, n * HF : (n + 1) * HF],
                                start=(c == 0),
                                stop=(c == NCH - 1),
                            )
                    bc_sb = attp.tile([P, S], F32, tag="bc", bufs=2, name="bc_sb")
                    for n in range(NHALF):
                        psc = pscs[n]
                        # 1/sums via the single-op Newton-Raphson DVE
                        # reciprocal (~51 ULP -- far below the fp32r matmul
                        # noise floor).  The exact DVE RECIPROCAL costs
                        # ~3.4us per row (iterative divide), and an ACT
                        # ln/exp pair thrashes the activation table set.
                        recip = attp.tile([HD + 1, HF], F32, tag="recip", bufs=2,
                                          name="recip")
                        # custom-DVE ops misbehave at a non-zero base
                        # partition on HW (uop indexing bug; verified:
                        # base-64 returns -sin(x) garbage).  Run over all
                        # 65 rows from base 0; only row 64 (the sums row)
                        # is consumed.
                        nc.vector.reciprocal_approx_fast(
                            recip[0 : HD + 1, :], psc[0 : HD + 1, :])
                        recip_r = attp.tile([HD + 1, HF], R, tag="recipr",
                                            bufs=2, name="recip_r")
                        nc.vector.tensor_copy(recip_r[HD : HD + 1, :],
                                              recip[HD : HD + 1, :])
                        psb = psA.tile([P, HF], F32, tag="ps", name="ps_bc")
                        nc.tensor.matmul(
                            psb[:, :],
                            ones_r[HD : HD + 1, :],
                            recip_r[HD : HD + 1, :],
                            start=True,
                            stop=True,
                        )
                        nc.vector.tensor_copy(bc_sb[:, n * HF : (n + 1) * HF],
                                              psb[:, :])
                        # normalized ctx^T slice for this head -> DRAM
                        ctx_sb = attp.tile([HD, HF], R, tag="ctxsb", bufs=2,
                                           name="ctx_sb")
                        nc.vector.tensor_mul(
                            ctx_sb[:, :],
                            psc[0:HD, :],
                            bc_sb[0:HD, n * HF : (n + 1) * HF],
                        )
                        nc.sync.dma_start(
                            ctx_dram[h * HD : (h + 1) * HD, n * HF : (n + 1) * HF],
                            ctx_sb[:, :],
                        )
                    for c in range(NCH):
                        pr = attp.tile([P, S], F32, tag="probs", bufs=3, name="pr")
                        eng = nc.gpsimd if c < n_gps else nc.vector
                        eng.tensor_mul(pr[:, :], exp_tiles[c].bitcast(F32)[:, :],
                                       bc_sb[:, :])
                        nc.sync.dma_start(probsT[h, c * P : (c + 1) * P, :], pr[:, :])
                    return nxt_tiles

                cur = [emit_scores_chunk(0, c) for c in range(NCH)]
                for h in range(H):
                    cur = emit_head(h, cur, h + 1 if h + 1 < H else None)

        # --- output projection: out[s, do] = ctx @ Wo + bo ---
        with ExitStack() as fin:
            outp = fin.enter_context(tc.tile_pool(name="outp", bufs=1))
            wot = [outp.tile([P, D], R, tag=f"wo{g}", name=f"wo{g}")
                   for g in range(NCH)]
            ctxp = [outp.tile([P, S], R, tag=f"ctxp{g}", name=f"ctxp{g}")
                    for g in range(NCH)]
            for g in range(NCH):
                nc.sync.dma_start(wot[g][:, :], wo[g * P : (g + 1) * P, :].bitcast(R))
                nc.sync.dma_start(ctxp[g][:, :], ctx_dram[g * P : (g + 1) * P, :])
            for i in range(NCH):
                o_sb = outp.tile([P, D], F32, tag="osb", bufs=3, name="o_sb")
                for n in range(NHALF):
                    ps = psA.tile([P, HF], F32, tag="ps", name="ps_out")
                    for g in range(NCH):
                        nc.tensor.matmul(
                            ps[:, :],
                            ctxp[g][:, i * P : (i + 1) * P],
                            wot[g][:, n * HF : (n + 1) * HF],
                            start=(g == 0),
                            stop=False,
                        )
                    nc.tensor.matmul(
                        ps[:, :],
                        ones_r[0:1, :],
                        bo_row[0:1, n * HF : (n + 1) * HF],
                        start=False,
                        stop=True,
                    )
                    nc.scalar.copy(o_sb[:, n * HF : (n + 1) * HF], ps[:, :])
                nc.sync.dma_start(out[i * P : (i + 1) * P, :], o_sb[:, :])


def build_program():
    nc = bacc.Bacc("TRN2", target_bir_lowering=False, debug=False)
    io = {
        "hsT": nc.dram_tensor("hsT", [D, S], F32, kind="ExternalInput").ap(),
        "Wq": nc.dram_tensor("Wq", [D, D], F32, kind="ExternalInput").ap(),
        "bq": nc.dram_tensor("bq", [D], F32, kind="ExternalInput").ap(),
        "Wk": nc.dram_tensor("Wk", [D, D], F32, kind="ExternalInput").ap(),
        "bk": nc.dram_tensor("bk", [D], F32, kind="ExternalInput").ap(),
        "Wv": nc.dram_tensor("Wv", [D, D], F32, kind="ExternalInput").ap(),
        "bv": nc.dram_tensor("bv", [D], F32, kind="ExternalInput").ap(),
        "Wo": nc.dram_tensor("Wo", [D, D], F32, kind="ExternalInput").ap(),
        "bo": nc.dram_tensor("bo", [D], F32, kind="ExternalInput").ap(),
        "out": nc.dram_tensor("out", [S, D], F32, kind="ExternalOutput").ap(),
        "probsT": nc.dram_tensor("probsT", [H, S, S], F32, kind="ExternalOutput").ap(),
    }
    with nc.allow_low_precision(reason="fp32r matmul input rounding"), \
            tile.TileContext(nc) as tc:
        _emit(tc, io)
    nc.compile()
    return nc


_program = None
last_exec_time_ns = None
last_mean_exec_time_ns = None


def _get_program():
    global _program
    if _program is None:
        _program = build_program()
    return _program


def kernel(hidden_states, Wq, bq, Wk, bk, Wv, bv, Wo, bo):
    """Full-input, full-output attention. Shards batch over 8 cores."""
    global last_exec_time_ns, last_mean_exec_time_ns
    hs = np.asarray(hidden_states, dtype=np.float32)
    shared = {
        "Wq": np.ascontiguousarray(np.asarray(Wq, np.float32)),
        "bq": np.ascontiguousarray(np.asarray(bq, np.float32)),
        "Wk": np.ascontiguousarray(np.asarray(Wk, np.float32)),
        "bk": np.ascontiguousarray(np.asarray(bk, np.float32)),
        "Wv": np.ascontiguousarray(np.asarray(Wv, np.float32)),
        "bv": np.ascontiguousarray(np.asarray(bv, np.float32)),
        "Wo": np.ascontiguousarray(np.asarray(Wo, np.float32)),
        "bo": np.ascontiguousarray(np.asarray(bo, np.float32)),
    }
    in_maps = [
        {"hsT": np.ascontiguousarray(hs[b].T), **shared} for b in range(NCORES)
    ]
    nc = _get_program()
    trace = os.environ.get("KERNEL_TRACE", "0") == "1"
    res = run_bass_kernel_spmd(nc, in_maps, core_ids=list(range(NCORES)),
                               trace=trace)
    last_exec_time_ns = res.exec_time_ns
    last_mean_exec_time_ns = res.mean_exec_time_ns

    out = np.empty((B, S, D), np.float32)
    probs = np.empty((B, H, S, S), np.float32)
    for b in range(NCORES):
        out[b] = res.results[b]["out"]
        probs[b] = res.results[b]["probsT"].transpose(0, 2, 1)
    return out, probs


# revision 17
# speedup vs baseline: 1.2145x; 1.0536x over previous
"""Multi-head attention Bass/Tile kernel for Trainium2, 8-core SPMD.

Problem: B=8, S=1024, D=1024, H=16 (head dim 64) attention that returns
both the attention output [B,S,D] and the softmax probabilities
[B,H,S,S].

Sharding: data-parallel over batch -- core b computes batch element b.

Per-core design (one batch element):
  - The host passes hs[b].T (``hsT`` [D,S]) so the Q/K projections can
    produce QT/KT in transposed [d, s] layout directly (the PE contracts
    over the partition dim of both operands, so hs always appears
    transposed; transposing on the host is free).
  - scores are computed transposed: scoresT[sk, sq] = K Q^T via
    lhsT=KT_h, rhs=QT_h (contraction over the 64-dim head axis).
  - exp on the scalar engine with the 1/sqrt(64) scale folded in.
  - V is kept in natural [sk, dv] layout with a ones column appended per
    head, so each context matmul (lhsT=V_h|1, rhs=expT_h) also produces
    the softmax denominators in psum row 64.
  - probs output is written to DRAM as [H, sk, sq] (contiguous stores);
    the host transposes to [H, sq, sk] while gathering (f32 DMA
    transpose does not exist on TRN2, and recomputing scores in the
    other orientation would double the scalar-engine exp work).
  - normalized ctx^T is staged through a DRAM scratch and re-read
    pair-packed so the out-projection runs K=128 matmuls and produces
    the attention output in natural [s, d] layout directly.
  - All matmuls run as float32r (full PE rate at free-dim >= 256).
    Walrus requires fp32r matmul operands to be *produced* rounded, so
    every matmul-feeding tile is dtype float32r (DMA loads bitcast the
    f32 DRAM side; ACT/DVE producers write f32r directly).
"""

import os

os.environ.setdefault("MYCRO_LOCAL_CACHE", "1")

from contextlib import ExitStack

import numpy as np

import concourse.bass as bass  # noqa: F401  (bass must import before tile)
import concourse.mybir as mybir
import concourse.tile as tile
from concourse import bacc
from concourse.bass_utils import run_bass_kernel_spmd

F32 = mybir.dt.float32
F32R = mybir.dt.float32r
AF = mybir.ActivationFunctionType

B, S, D, H, HD = 8, 1024, 1024, 16, 64
P = 128
NCH = D // P  # 8 chunks of 128 along d or s
HF = 512  # fp32 psum bank free-dim limit
NHALF = S // HF  # 2
NCORES = 8


def _emit(tc, io):
    nc = tc.nc
    hsT, wq, bq, wk, bk, wv, bv, wo, bo = (
        io["hsT"], io["Wq"], io["bq"], io["Wk"], io["bk"],
        io["Wv"], io["bv"], io["Wo"], io["bo"],
    )
    out, probsT = io["out"], io["probsT"]
    R = F32R

    with ExitStack() as top:
        const = top.enter_context(tc.tile_pool(name="const", bufs=1))
        ones_f = const.tile([P, P], F32, tag="ones_f", name="ones_f")
        nc.vector.memset(ones_f[:, :], 1.0)
        # fp32r view of ones for rank-1 bias / broadcast matmuls
        # (memset cannot write f32r directly; a DVE copy can)
        ones_r = const.tile([P, P], R, tag="ones_r", name="ones_r")
        nc.vector.tensor_copy(ones_r[:, :], ones_f[:, :])
        bq_sb = const.tile([P, NCH], F32, tag="bq_sb", name="bq_sb")
        nc.sync.dma_start(bq_sb[:, :], bq.rearrange("(c p) -> p c", p=P))
        bk_sb = const.tile([P, NCH], F32, tag="bk_sb", name="bk_sb")
        nc.sync.dma_start(bk_sb[:, :], bk.rearrange("(c p) -> p c", p=P))
        bv_row = const.tile([1, D], R, tag="bv_row", name="bv_row")
        nc.sync.dma_start(bv_row[:, :], bv.rearrange("(o d) -> o d", o=1).bitcast(R))
        bo_row = const.tile([1, D], R, tag="bo_row", name="bo_row")
        nc.sync.dma_start(bo_row[:, :], bo.rearrange("(o d) -> o d", o=1).bitcast(R))

        # PSUM: one 4-buffer [128,512] pool shared by the QKV chains, ctx
        # accumulators, bc broadcasts and the out-projection (deep rotation
        # keeps PE accumulation chains back-to-back so HAM reaches full
        # clock), plus a 2-buffer [128,1024] pool for scores.  4 + 4 = 8
        # banks.
        psA = top.enter_context(tc.tile_pool(name="psA", bufs=4, space="PSUM"))
        ps_sc = top.enter_context(tc.tile_pool(name="ps_sc", bufs=2, space="PSUM"))

        # Normalized ctx^T staged through DRAM ([dc, s]); re-read in the
        # out-projection as pair-packed [128, S] chunks (keeps SBUF small
        # and the out-proj matmuls at K=128).
        ctx_dram = nc.dram_tensor("ctx_scratch", [D, S], R, kind="Internal").ap()

        with ExitStack() as mid:
            qkvp = mid.enter_context(tc.tile_pool(name="qkvp", bufs=1))
            vt = [qkvp.tile([P, H, HD + 1], R, tag=f"v{i}", name=f"v{i}")
                  for i in range(NCH)]
            for i in range(NCH):
                # ones column per head (f32r via DVE copy from f32 ones)
                nc.vector.tensor_copy(
                    vt[i][:, :, HD : HD + 1],
                    ones_f[:, 0:H].rearrange("p (h o) -> p h o", o=1),
                )
            qktp = mid.enter_context(tc.tile_pool(name="qktp", bufs=1))
            qt = [None] * NCH
            kt = [None] * NCH

            with ExitStack() as ld:
                ldp = ld.enter_context(tc.tile_pool(name="ldp", bufs=1))
                hst = []
                for c in range(NCH):
                    t = ldp.tile([P, S], R, tag=f"hst{c}", bufs=1, name=f"hst{c}")
                    nc.sync.dma_start(t[:, :], hsT[c * P : (c + 1) * P, :].bitcast(R))
                    hst.append(t)

                # --- V projection: V[sk, dv] = hs @ Wv + bv ---
                wv_r = wv.rearrange("(c p) d -> p c d", p=P).bitcast(R)
                for n in range(NHALF):
                    wvt = ldp.tile([P, NCH, HF], R, tag="wv", bufs=1, name="wvt")
                    nc.sync.dma_start(wvt[:, :, :], wv_r[:, :, n * HF : (n + 1) * HF])
                    for i in range(NCH):
                        ps = psA.tile([P, HF], F32, tag="ps", name="ps_v")
                        for c in range(NCH):
                            nc.tensor.matmul(
                                ps[:, :],
                                hst[c][:, i * P : (i + 1) * P],
                                wvt[:, c, :],
                                start=(c == 0),
                                stop=False,
                            )
                        # bias as a rank-1 (ones ⊗ bv) accumulation
                        nc.tensor.matmul(
                            ps[:, :],
                            ones_r[0:1, :],
                            bv_row[0:1, n * HF : (n + 1) * HF],
                            start=False,
                            stop=True,
                        )
                        # evict on ScalarE (keeps VectorE free for the
                        # attention-phase normalize work)
                        nc.scalar.copy(
                            vt[i][:, n * 8 : (n + 1) * 8, 0:HD],
                            ps.rearrange("p (h e) -> p h e", e=HD),
                        )

                # --- Q/K projections into transposed [do, s] layout ---
                wq_r = wq.rearrange("(c p) (j q) -> p c j q", p=P, q=P).bitcast(R)
                wk_r = wk.rearrange("(c p) (j q) -> p c j q", p=P, q=P).bitcast(R)
                for j in range(NCH):
                    for (w_r, b_sb, dst, tag) in (
                        (wq_r, bq_sb, qt, "qtile"),
                        (wk_r, bk_sb, kt, "ktile"),
                    ):
                        wjt = ldp.tile([P, NCH, P], R, tag=f"wj_{tag}",
                                       bufs=2, name="wjt")
                        nc.sync.dma_start(wjt[:, :, :], w_r[:, :, j, :])
                        dtile = qktp.tile([P, S], R, tag=f"{tag}{j}",
                                          name=f"{tag}{j}")
                        dst[j] = dtile
                        for n in range(NHALF):
                            ps = psA.tile([P, HF], F32, tag="ps", name="ps_qk")
                            for c in range(NCH):
                                nc.tensor.matmul(
                                    ps[:, :],
                                    wjt[:, c, :],
                                    hst[c][:, n * HF : (n + 1) * HF],
                                    start=(c == 0),
                                    stop=(c == NCH - 1),
                                )
                            nc.scalar.activation(
                                dtile[:, n * HF : (n + 1) * HF],
                                ps[:, :],
                                AF.Identity,
                                bias=b_sb[:, j : j + 1],
                                scale=1.0,
                            )

            # --- attention, software-pipelined over heads ---
            # The scores matmuls of head h+1 are interleaved chunk-by-chunk
            # with the ctx matmuls of head h so the tensor engine's stalls
            # stay short (HAM keeps the PE at full clock), while ScalarE
            # runs exp and VectorE/GpSimd split the probs normalization.
            n_gps = int(os.environ.get("GPSIMD_CHUNKS", "3"))
            with ExitStack() as att:
                attp = att.enter_context(tc.tile_pool(name="attp", bufs=1))

                def emit_scores_pair_chunk(g, c):
                    """scores+exp for chunk c of BOTH heads of pair g.
                    The two K=64 matmuls sit in different PE row groups
                    (base partition 0 / 64) and are emitted adjacently, so
                    they execute concurrently in the array."""
                    ps_a = ps_sc.tile([P, S], F32, tag="sc", name="ps_sca")
                    ps_b = ps_sc.tile([P, S], F32, tag="sc", name="ps_scb")
                    for n in range(NHALF):
                        nc.tensor.matmul(
                            ps_a[:, n * HF : (n + 1) * HF],
                            kt[g][0:HD, c * P : (c + 1) * P],
                            qt[g][0:HD, n * HF : (n + 1) * HF],
                            start=True,
                            stop=True,
                        )
                        nc.tensor.matmul(
                            ps_b[:, n * HF : (n + 1) * HF],
                            kt[g][HD : 2 * HD, c * P : (c + 1) * P],
                            qt[g][HD : 2 * HD, n * HF : (n + 1) * HF],
                            start=True,
                            stop=True,
                        )
                    e_a = attp.tile([P, S], R, tag="expt", bufs=17, name="expta")
                    nc.scalar.activation(e_a[:, :], ps_a[:, :], AF.Exp, scale=0.125)
                    e_b = attp.tile([P, S], R, tag="expt", bufs=17, name="exptb")
                    nc.scalar.activation(e_b[:, :], ps_b[:, :], AF.Exp, scale=0.125)
                    return e_a, e_b

                def emit_head(h, exp_tiles, nxt):
                    """ctx+normalize for head h, interleaving the paired
                    scores matmuls of pair ``nxt`` between ctx chunks."""
                    nxt_tiles = []
                    pscs = [psA.tile([P, HF], F32, tag="ps",
                                     name="ps_ctx") for _ in range(NHALF)]
                    for c in range(NCH):
                        if nxt is not None:
                            nxt_tiles.append(emit_scores_pair_chunk(nxt, c))
                        for n in range(NHALF):
                            nc.tensor.matmul(
                                pscs[n][0 : HD + 1, :],
                                vt[c][:, h, :],
                                exp_tiles[c][:, n * HF : (n + 1) * HF],
                                start=(c == 0),
                                stop=(c == NCH - 1),
                            )
                    bc_sb = attp.tile([P, S], F32, tag="bc", bufs=2, name="bc_sb")
                    for n in range(NHALF):
                        psc = pscs[n]
                        # 1/sums via the single-op Newton-Raphson DVE
                        # reciprocal (~51 ULP -- far below the fp32r matmul
                        # noise floor).  The exact DVE RECIPROCAL costs
                        # ~3.4us per row (iterative divide), and an ACT
                        # ln/exp pair thrashes the activation table set.
                        recip = attp.tile([HD + 1, HF], F32, tag="recip", bufs=2,
                                          name="recip")
                        # custom-DVE ops misbehave at a non-zero base
                        # partition on HW (uop indexing bug; verified:
                        # base-64 returns -sin(x) garbage).  Run over all
                        # 65 rows from base 0; only row 64 (the sums row)
                        # is consumed.
                        nc.vector.reciprocal_approx_fast(
                            recip[0 : HD + 1, :], psc[0 : HD + 1, :])
                        recip_r = attp.tile([HD + 1, HF], R, tag="recipr",
                                            bufs=2, name="recip_r")
                        nc.vector.tensor_copy(recip_r[HD : HD + 1, :],
                                              recip[HD : HD + 1, :])
                        psb = psA.tile([P, HF], F32, tag="ps", name="ps_bc")
                        nc.tensor.matmul(
                            psb[:, :],
                            ones_r[HD : HD + 1, :],
                            recip_r[HD : HD + 1, :],
                            start=True,
                            stop=True,
                        )
                        nc.vector.tensor_copy(bc_sb[:, n * HF : (n + 1) * HF],
                                              psb[:, :])
                        # normalized ctx^T slice for this head -> DRAM
                        ctx_sb = attp.tile([HD, HF], R, tag="ctxsb", bufs=2,
                                           name="ctx_sb")
                        nc.vector.tensor_mul(
                            ctx_sb[:, :],
                            psc[0:HD, :],
                            bc_sb[0:HD, n * HF : (n + 1) * HF],
                        )
                        nc.sync.dma_start(
                            ctx_dram[h * HD : (h + 1) * HD, n * HF : (n + 1) * HF],
                            ctx_sb[:, :],
                        )
                    for c in range(NCH):
                        pr = attp.tile([P, S], F32, tag="probs", bufs=3, name="pr")
                        eng = nc.gpsimd if c < n_gps else nc.vector
                        eng.tensor_mul(pr[:, :], exp_tiles[c].bitcast(F32)[:, :],
                                       bc_sb[:, :])
                        nc.sync.dma_start(probsT[h, c * P : (c + 1) * P, :], pr[:, :])
                    return nxt_tiles

                cur = [emit_scores_pair_chunk(0, c) for c in range(NCH)]
                for g in range(H // 2):
                    nxt = g + 1 if g + 1 < H // 2 else None
                    nxt_tiles = emit_head(2 * g, [ab[0] for ab in cur], nxt)
                    emit_head(2 * g + 1, [ab[1] for ab in cur], None)
                    cur = nxt_tiles

        # --- output projection: out[s, do] = ctx @ Wo + bo ---
        with ExitStack() as fin:
            outp = fin.enter_context(tc.tile_pool(name="outp", bufs=1))
            wot = [outp.tile([P, D], R, tag=f"wo{g}", name=f"wo{g}")
                   for g in range(NCH)]
            ctxp = [outp.tile([P, S], R, tag=f"ctxp{g}", name=f"ctxp{g}")
                    for g in range(NCH)]
            for g in range(NCH):
                nc.sync.dma_start(wot[g][:, :], wo[g * P : (g + 1) * P, :].bitcast(R))
                nc.sync.dma_start(ctxp[g][:, :], ctx_dram[g * P : (g + 1) * P, :])
            for i in range(NCH):
                o_sb = outp.tile([P, D], F32, tag="osb", bufs=3, name="o_sb")
                for n in range(NHALF):
                    ps = psA.tile([P, HF], F32, tag="ps", name="ps_out")
                    for g in range(NCH):
                        nc.tensor.matmul(
                            ps[:, :],
                            ctxp[g][:, i * P : (i + 1) * P],
                            wot[g][:, n * HF : (n + 1) * HF],
                            start=(g == 0),
                            stop=False,
                        )
                    nc.tensor.matmul(
                        ps[:, :],
                        ones_r[0:1, :],
                        bo_row[0:1, n * HF : (n + 1) * HF],
                        start=False,
                        stop=True,
                    )
                    nc.scalar.copy(o_sb[:, n * HF : (n + 1) * HF], ps[:, :])
                nc.sync.dma_start(out[i * P : (i + 1) * P, :], o_sb[:, :])


def build_program():
    nc = bacc.Bacc("TRN2", target_bir_lowering=False, debug=False)
    io = {
        "hsT": nc.dram_tensor("hsT", [D, S], F32, kind="ExternalInput").ap(),
        "Wq": nc.dram_tensor("Wq", [D, D], F32, kind="ExternalInput").ap(),
        "bq": nc.dram_tensor("bq", [D], F32, kind="ExternalInput").ap(),
        "Wk": nc.dram_tensor("Wk", [D, D], F32, kind="ExternalInput").ap(),
        "bk": nc.dram_tensor("bk", [D], F32, kind="ExternalInput").ap(),
        "Wv": nc.dram_tensor("Wv", [D, D], F32, kind="ExternalInput").ap(),
        "bv": nc.dram_tensor("bv", [D], F32, kind="ExternalInput").ap(),
        "Wo": nc.dram_tensor("Wo", [D, D], F32, kind="ExternalInput").ap(),
        "bo": nc.dram_tensor("bo", [D], F32, kind="ExternalInput").ap(),
        "out": nc.dram_tensor("out", [S, D], F32, kind="ExternalOutput").ap(),
        "probsT": nc.dram_tensor("probsT", [H, S, S], F32, kind="ExternalOutput").ap(),
    }
    with nc.allow_low_precision(reason="fp32r matmul input rounding"), \
            tile.TileContext(nc) as tc:
        _emit(tc, io)
    nc.compile()
    return nc


_program = None
last_exec_time_ns = None
last_mean_exec_time_ns = None


def _get_program():
    global _program
    if _program is None:
        _program = build_program()
    return _program


def kernel(hidden_states, Wq, bq, Wk, bk, Wv, bv, Wo, bo):
    """Full-input, full-output attention. Shards batch over 8 cores."""
    global last_exec_time_ns, last_mean_exec_time_ns
    hs = np.asarray(hidden_states, dtype=np.float32)
    shared = {
        "Wq": np.ascontiguousarray(np.asarray(Wq, np.float32)),
        "bq": np.ascontiguousarray(np.asarray(bq, np.float32)),
        "Wk": np.ascontiguousarray(np.asarray(Wk, np.float32)),
        "bk": np.ascontiguousarray(np.asarray(bk, np.float32)),
        "Wv": np.ascontiguousarray(np.asarray(Wv, np.float32)),
        "bv": np.ascontiguousarray(np.asarray(bv, np.float32)),
        "Wo": np.ascontiguousarray(np.asarray(Wo, np.float32)),
        "bo": np.ascontiguousarray(np.asarray(bo, np.float32)),
    }
    in_maps = [
        {"hsT": np.ascontiguousarray(hs[b].T), **shared} for b in range(NCORES)
    ]
    nc = _get_program()
    trace = os.environ.get("KERNEL_TRACE", "0") == "1"
    res = run_bass_kernel_spmd(nc, in_maps, core_ids=list(range(NCORES)),
                               trace=trace)
    last_exec_time_ns = res.exec_time_ns
    last_mean_exec_time_ns = res.mean_exec_time_ns

    out = np.empty((B, S, D), np.float32)
    probs = np.empty((B, H, S, S), np.float32)
    for b in range(NCORES):
        out[b] = res.results[b]["out"]
        probs[b] = res.results[b]["probsT"].transpose(0, 2, 1)
    return out, probs
